# revision 1
# baseline (speedup 1.0000x reference)
"""GATr model Bass kernel for 8 TRN2 NeuronCores.

Sharding: core i handles batch i//2, query-half i%2 (2048 of 4096 tokens).
Per layer: LN + QKV projections run on the full sequence (so K/V are local),
attention runs for the core's own query half (flash-style over 32 kv chunks,
no-max-subtraction softmax with a ones-column producing denominators), MLP
runs on the own half, then an AllGather over the core pair rebuilds the full
hidden state for the next layer.

Internal layout: multivector blades in RAW-MASK order (bit0=e0..bit3=e3),
feature index f = channel*16 + blade (80 features), activations kept
feature-major [80, n] except where the geometric product needs token-major.
All equi_linear maps are host-expanded to dense [80, 80] matrices; the
attention INNER-product mask and 1/sqrt(16H) scale are folded into Wk; the
geometric product uses XOR-indexed access-pattern views with a separable sign
table. Matmuls run as float32r.
"""

import numpy as np

INV, VM, H, NBLK = 32, 1, 5, 3
B, N = 4, 4096
NOWN = N // 2          # tokens per core (query half)
NF = H * 16            # 80 features
METRIC = (0.0, 1.0, 1.0, 1.0)
T_RAW = (3, 5, 9)      # e01, e02, e03 raw blade masks
GRADE_RAW = [bin(m).count("1") for m in range(16)]


# ---------------------------------------------------------------- host math
def _blade_mul(a, b):
    swaps = sum(bin(a >> (i + 1)).count("1") for i in range(4) if (b >> i) & 1)
    coef = -1.0 if swaps % 2 else 1.0
    for i in range(4):
        if ((a & b) >> i) & 1:
            coef *= METRIC[i]
    return a ^ b, coef


def _c_raw():
    C = np.zeros((16, 16, 16), dtype=np.float32)
    for i in range(16):
        for j in range(16):
            k, c = _blade_mul(i, j)
            if c != 0.0:
                C[i, j, k] = c
    return C


def _expand_equi(w, n_in, n_out):
    W = np.zeros((n_in * 16, n_out * 16), dtype=np.float32)
    for c in range(16):
        W[c::16, c::16] = w[GRADE_RAW[c]].T
    return W


def _build_consts(w_in, w_out, wq, wk, wv, wo, w_mlp1, w_mlp2):
    consts = {}
    inner = np.array([0.0 if (m & 1) else 1.0 for m in range(16)], np.float32)
    mask80 = np.tile(inner, H)
    scale = 1.0 / np.sqrt(16.0 * H)

    wemb = np.zeros((36, NF), np.float32)
    for o in range(H):
        for i in range(1, INV):
            wemb[i, o * 16] = w_in[0, o, i]
        wemb[35, o * 16] = w_in[0, o, 0]
        for d, c in enumerate(T_RAW):
            wemb[32 + d, o * 16 + c] = w_in[2, o, 0]
    consts["wemb"] = np.ascontiguousarray(wemb[:35])
    consts["bemb"] = np.ascontiguousarray(wemb[35:36].T)  # [80, 1] bias

    wsel = np.zeros((NF, 35), np.float32)
    for j in range(INV):
        for i in range(H):
            wsel[i * 16, j] = w_out[0, j, i]
    for d, c in enumerate(T_RAW):
        for i in range(H):
            wsel[i * 16 + c, 32 + d] = w_out[2, 0, i]
    consts["wsel"] = wsel

    for l in range(NBLK):
        consts[f"wq{l}"] = _expand_equi(wq[l], H, H)
        consts[f"wk{l}"] = _expand_equi(wk[l], H, H) * (mask80[None, :] * scale)
        consts[f"wv{l}"] = _expand_equi(wv[l], H, H)
        consts[f"wo{l}"] = _expand_equi(wo[l], H, H)
        w1 = _expand_equi(w_mlp1[l], H, 2 * H)   # [80, 160]
        consts[f"w1a{l}"] = np.ascontiguousarray(w1[:, :NF])
        consts[f"w1b{l}"] = np.ascontiguousarray(w1[:, NF:])
        consts[f"w2{l}"] = _expand_equi(w_mlp2[l], H, H)

    C = _c_raw()
    signs = np.zeros((16, 16), np.float32)          # S[i, k] = C[i, k^i, k]
    for i in range(16):
        for k in range(16):
            signs[i, k] = C[i, k ^ i, k]
    consts["signs"] = np.ascontiguousarray(
        np.broadcast_to(signs.reshape(1, 256), (128, 256))
    )
    consts["lnmask"] = np.ascontiguousarray(mask80.reshape(NF, 1))
    return consts


# ---------------------------------------------------------------- bass build
_BUILT = None


def _split_excess_waits(nc, max_waits=1):
    """walrus setupSyncWait rejects >1 sem wait on one instruction; move
    excess waits onto same-engine carrier nops placed just before."""
    import bass_rust

    for bb in nc.main_func.blocks:
        il = bb.instructions
        i = 0
        while i < len(il):
            ins = il[i]
            si = ins.sync_info
            if si is not None and si.on_wait and len(si.on_wait) > max_waits:
                waits = list(si.on_wait)
                si.on_wait = waits[:max_waits]
                excess = waits[max_waits:]
                carriers = []
                for j in range(0, len(excess), max_waits):
                    nop = nc.engines[ins.engine].nop(nofuse=True)
                    nop.ins.sync_info = bass_rust.SyncInfo(
                        on_wait=excess[j : j + max_waits], on_update=[]
                    )
                    carriers.append(nop.ins)
                for bb2 in nc.main_func.blocks:
                    il2 = bb2.instructions
                    for c in carriers:
                        for k in range(len(il2) - 1, -1, -1):
                            if il2[k].name == c.name:
                                il2.pop(k)
                i = next(k for k, x in enumerate(il) if x.name == ins.name)
                for c in reversed(carriers):
                    il.insert(i, c)
                i += len(carriers)
            i += 1


def _build_nc():
    import concourse.bass as bass
    import concourse.mybir as mybir
    import concourse.tile as tile
    from concourse import bacc
    from concourse.bass import ds
    from concourse.masks import make_identity

    f32 = mybir.dt.float32
    f32r = mybir.dt.float32r
    AF = mybir.ActivationFunctionType
    ALU = mybir.AluOpType

    nc = bacc.Bacc("TRN2", target_bir_lowering=False, debug=False, num_devices=8)

    x_in = nc.declare_dram_parameter("x", [N, 35], f32, isOutput=False)
    out_p = nc.declare_dram_parameter("out", [NOWN, 35], f32, isOutput=True)
    wnames = ["wemb", "bemb", "wsel", "signs", "lnmask"]
    for l in range(NBLK):
        wnames += [f"wq{l}", f"wk{l}", f"wv{l}", f"wo{l}",
                   f"w1a{l}", f"w1b{l}", f"w2{l}"]
    shapes = {"wemb": [35, NF], "bemb": [NF, 1], "wsel": [NF, 35],
              "signs": [128, 256], "lnmask": [NF, 1]}
    wp = {
        name: nc.declare_dram_parameter(
            name, shapes.get(name, [NF, NF]), f32, isOutput=False
        )
        for name in wnames
    }

    def R(ap):
        # fp32r measured at tf32-class precision (mean rel 8e-4) — this
        # network amplifies that to ~3x output rms. Full fp32 matmuls.
        return ap

    with tile.TileContext(nc) as tc:
        pid = nc.partition_id()
        q0 = (pid % 2) * NOWN

        cst_cm = tc.tile_pool(name="cst", bufs=1)
        cst = cst_cm.__enter__()
        idn = cst.tile([128, 128], f32, name="idn")
        make_identity(nc, idn[:])
        ones_c = cst.tile([NF, 1], f32, name="ones_c")
        nc.vector.memset(ones_c[:], 1.0)
        ones_r = cst.tile([1, NF], f32, name="ones_r")
        nc.vector.memset(ones_r[:], 1.0)
        eps_t = cst.tile([1, 1], f32, name="eps_t")
        nc.vector.memset(eps_t[:], 1e-6)
        wsb = {}
        for name in wnames:
            t = cst.tile(shapes.get(name, [NF, NF]), f32, name=f"sb_{name}")
            nc.sync.dma_start(t[:], wp[name][:])
            wsb[name] = t

        sb_cm = tc.tile_pool(name="sb", bufs=1)
        sb = sb_cm.__enter__()

        # ---------------- embedding: x -> hT [80, 4096] feature-major
        hT = sb.tile([NF, N], f32, name="hT", tag="hT", bufs=1)
        with tc.tile_pool(name="pemb", bufs=2, space="PSUM") as pemb, \
             tc.tile_pool(name="sbemb", bufs=3) as sbemb:
            xT = sbemb.tile([35, N], f32, name="xT", bufs=1)
            for c in range(N // 128):
                xa = sbemb.tile([128, 35], f32, name="xa", bufs=3)
                nc.sync.dma_start(xa[:], x_in[c * 128:(c + 1) * 128, :])
                pxt = pemb.tile([35, 128], f32, name="pxt", bufs=2)
                nc.tensor.transpose(pxt[:], xa[:], idn[:])
                nc.vector.tensor_copy(xT[0:35, c * 128:(c + 1) * 128], pxt[:])
            for s in range(N // 512):
                pe = pemb.tile([NF, 512], f32, name="pe", bufs=2)
                nc.tensor.matmul(pe[:], R(wsb["wemb"][:]),
                                 R(xT[:, s * 512:(s + 1) * 512]),
                                 start=True, stop=True)
                nc.vector.tensor_scalar_add(hT[:, s * 512:(s + 1) * 512],
                                            pe[:], wsb["bemb"][:])

        # ---------------- layer norm helper (feature-major, slice loop)
        def layer_norm(src, n, name):
            """src [80, n] SBUF -> hln [80, n] SBUF"""
            hln = sb.tile([NF, n], f32, name=f"hln_{name}", tag=f"hln{n}", bufs=1)
            with tc.tile_pool(name=f"pln_{name}", bufs=2, space="PSUM") as pln, \
                 tc.tile_pool(name=f"sln_{name}", bufs=2) as sln:
                for s in range(n // 512):
                    sl = slice(s * 512, (s + 1) * 512)
                    hm = sln.tile([NF, 512], f32, name="hm", bufs=2)
                    nc.vector.tensor_scalar_mul(hm[:], src[:, sl], wsb["lnmask"][:])
                    sq = sln.tile([NF, 512], f32, name="sq", bufs=2)
                    nc.vector.tensor_tensor(sq[:], hm[:], src[:, sl], ALU.mult)
                    pip = pln.tile([1, 512], f32, name="pip", bufs=2)
                    nc.tensor.matmul(pip[:], R(ones_c[:]), R(sq[:]),
                                     start=True, stop=True)
                    sd = sln.tile([1, 512], f32, name="sd", bufs=2)
                    nc.scalar.activation(sd[:], pip[:], AF.Sqrt,
                                         bias=eps_t[:], scale=1.0 / float(NF))
                    rs = sln.tile([1, 512], f32, name="rs", bufs=2)
                    nc.vector.reciprocal(rs[:], sd[:])
                    pbc = pln.tile([NF, 512], f32, name="pbc", bufs=2)
                    nc.tensor.matmul(pbc[:], R(ones_r[:]), R(rs[:]),
                                     start=True, stop=True)
                    nc.vector.tensor_tensor(hln[:, sl], src[:, sl], pbc[:], ALU.mult)
            return hln

        # ---------------- layers
        h_cur = hT
        for l in range(NBLK):
            hln1 = layer_norm(h_cur, N, f"a{l}")

            # own-half views of h and ln(h), materialized via dynamic-offset DMA
            hq = sb.tile([NF, NOWN], f32, name="hq", tag="hq", bufs=1)
            nc.sync.dma_start(hq[:], h_cur[:, ds(q0, NOWN)])
            hlnq = sb.tile([NF, NOWN], f32, name="hlnq", tag="hlnq", bufs=1)
            nc.sync.dma_start(hlnq[:], hln1[:, ds(q0, NOWN)])

            # QKV projections (feature-major; V also needs token-major chunks)
            qT = sb.tile([NF, NOWN], f32, name="qT", bufs=1)
            kT = sb.tile([NF, N], f32, name="kT", bufs=1)
            vtok = sb.tile([128, 32 * 81], f32, name="vtok", bufs=1)
            nc.vector.memset(
                vtok[:].rearrange("p (c u) -> p c u", c=32, u=81)[:, :, 80:81], 1.0
            )
            with tc.tile_pool(name="pqkv", bufs=2, space="PSUM") as pqkv:
                for s in range(NOWN // 512):
                    pq = pqkv.tile([NF, 512], f32, name="pq", bufs=2)
                    nc.tensor.matmul(pq[:], R(wsb[f"wq{l}"][:]),
                                     R(hlnq[:, s * 512:(s + 1) * 512]),
                                     start=True, stop=True)
                    nc.vector.tensor_copy(qT[:, s * 512:(s + 1) * 512], pq[:])
                for s in range(N // 512):
                    sl = slice(s * 512, (s + 1) * 512)
                    pk = pqkv.tile([NF, 512], f32, name="pk", bufs=2)
                    nc.tensor.matmul(pk[:], R(wsb[f"wk{l}"][:]), R(hln1[:, sl]),
                                     start=True, stop=True)
                    nc.scalar.copy(kT[:, sl], pk[:])
                # V token-major directly: lhsT = hln1 chunk, rhs = Wv
                for c in range(32):
                    pv = pqkv.tile([128, NF], f32, name="pv", bufs=2)
                    nc.tensor.matmul(pv[:], R(hln1[:, c * 128:(c + 1) * 128]),
                                     R(wsb[f"wv{l}"][:]), start=True, stop=True)
                    nc.vector.tensor_copy(vtok[:, c * 81:c * 81 + 80], pv[:])

            # attention: 32 kv chunks, flash accumulation into psAV [81, 2048]
            avs = sb.tile([81, NOWN], f32, name="avs", bufs=1)
            with tc.tile_pool(name="pS", bufs=2, space="PSUM") as pS, \
                 tc.tile_pool(name="pAV", bufs=1, space="PSUM") as pAV, \
                 tc.tile_pool(name="sP", bufs=3) as sP:
                psAV = pAV.tile([81, NOWN], f32, name="psAV")
                for c in range(32):
                    kch = kT[:, c * 128:(c + 1) * 128]
                    vch = vtok[:, c * 81:c * 81 + 81]
                    for ns in range(NOWN // 1024):
                        psS = pS.tile([128, 1024], f32, name="psS", bufs=2)
                        for j in range(2):
                            nc.tensor.matmul(
                                psS[:, j * 512:(j + 1) * 512], R(kch),
                                R(qT[:, ns * 1024 + j * 512:ns * 1024 + (j + 1) * 512]),
                                start=True, stop=True)
                        pT = sP.tile([128, 1024], f32, name="pT", bufs=3)
                        nc.scalar.activation(pT[:], psS[:], AF.Exp)
                        for j in range(2):
                            nc.tensor.matmul(
                                psAV[:, ns * 1024 + j * 512:ns * 1024 + (j + 1) * 512],
                                R(vch), R(pT[:, j * 512:(j + 1) * 512]),
                                start=(c == 0), stop=(c == 31))
                nc.vector.tensor_copy(avs[:], psAV[:])

            # post-attention: wo, divide by denominator, residual
            dnm = sb.tile([1, NOWN], f32, name="dnm", bufs=1)
            nc.sync.dma_start(dnm[:], avs[80:81, :])
            rd = sb.tile([1, NOWN], f32, name="rd", bufs=1)
            nc.vector.reciprocal(rd[:], dnm[:])
            h_att = sb.tile([NF, NOWN], f32, name="h_att", bufs=1)
            with tc.tile_pool(name="ppost", bufs=2, space="PSUM") as ppost, \
                 tc.tile_pool(name="spost", bufs=2) as spost:
                for s in range(NOWN // 512):
                    sl = slice(s * 512, (s + 1) * 512)
                    pw = ppost.tile([NF, 512], f32, name="pw", bufs=2)
                    nc.tensor.matmul(pw[:], R(wsb[f"wo{l}"][:]), R(avs[:NF, sl]),
                                     start=True, stop=True)
                    pb = ppost.tile([NF, 512], f32, name="pb", bufs=2)
                    nc.tensor.matmul(pb[:], R(ones_r[:]), R(rd[:, sl]),
                                     start=True, stop=True)
                    pbs = spost.tile([NF, 512], f32, name="pbs", bufs=2)
                    nc.scalar.copy(pbs[:], pb[:])
                    t1 = spost.tile([NF, 512], f32, name="t1", bufs=2)
                    nc.vector.tensor_tensor(t1[:], pw[:], pbs[:], ALU.mult)
                    nc.vector.tensor_tensor(h_att[:, sl], t1[:], hq[:, sl], ALU.add)

            # MLP on own half
            hln2 = layer_norm(h_att, NOWN, f"m{l}")
            ltok = sb.tile([128, 16 * NF], f32, name="ltok", bufs=1)
            rtok = sb.tile([128, 16 * NF], f32, name="rtok", bufs=1)
            with tc.tile_pool(name="plr", bufs=2, space="PSUM") as plr:
                for c in range(16):
                    lhs = R(hln2[:, c * 128:(c + 1) * 128])
                    pl = plr.tile([128, NF], f32, name="pl", bufs=2)
                    nc.tensor.matmul(pl[:], lhs, R(wsb[f"w1a{l}"][:]),
                                     start=True, stop=True)
                    nc.vector.tensor_copy(ltok[:, c * NF:(c + 1) * NF], pl[:])
                    pr = plr.tile([128, NF], f32, name="pr", bufs=2)
                    nc.tensor.matmul(pr[:], lhs, R(wsb[f"w1b{l}"][:]),
                                     start=True, stop=True)
                    nc.vector.tensor_copy(rtok[:, c * NF:(c + 1) * NF], pr[:])

            # geometric product via XOR views. APs are capped at 5-D, so the
            # 4 blade bits are grouped into maximal same-flip runs (a flipped
            # run of bits == reversal of that grouped dim); (chunk, channel)
            # merges into one uniform dim of 80 (step 16).
            gp = sb.tile([128, 16 * NF], f32, name="gp", bufs=1)
            lsb_t = sb.tile([128, 16 * NF], f32, name="lsb_t", tag="gpscr", bufs=2)
            trm = sb.tile([128, 16 * NF], f32, name="trm", tag="gpscr", bufs=2)
            NCH = 16 * H  # 80 merged (chunk, channel) blocks

            def bit_runs(i, bits):
                runs = []
                for b in bits:
                    f = (i >> b) & 1
                    if runs and runs[-1][1] == f:
                        runs[-1][0] *= 2
                    else:
                        runs.append([2, f])
                return runs  # list of [size, flip], msb-first

            def gp_emit(i, koff, ksz):
                # one TT-triple over blade sub-range [koff, koff+ksz). DVE APs
                # allow only 3 free dims (ck + 2 bit-runs) — recurse otherwise.
                bits = list(range(ksz.bit_length() - 2, -1, -1))
                runs = bit_runs(i, bits)
                if len(runs) > 2:
                    gp_emit(i, koff, ksz // 2)
                    gp_emit(i, koff + ksz // 2, ksz // 2)
                    return
                sizes = [r[0] for r in runs]
                names = [f"g{j}" for j in range(len(sizes))]
                pat = f"p ck ({' '.join(names)}) -> p ck {' '.join(names)}"
                kw = dict(zip(names, sizes))

                def natview(t, off):
                    v = t.rearrange("p (ck k) -> p ck k", ck=NCH, k=16)
                    return v[:, :, off:off + ksz].rearrange(pat, **kw)

                gv = natview(gp[:], koff)
                sv = (wsb["signs"][:, i * 16:(i + 1) * 16].unsqueeze(1)
                      .broadcast_to([128, NCH, 16])[:, :, koff:koff + ksz]
                      .rearrange(pat, **kw))
                # r view: XOR image of the out-blade range, grouped, with
                # flipped runs realized as dim reversals
                start = (koff ^ i) & (0xF ^ (ksz - 1))
                rv = natview(rtok[:], start)
                for j, (_, f) in enumerate(runs):
                    if f:
                        idx = [slice(None)] * (2 + len(runs))
                        idx[2 + j] = slice(None, None, -1)
                        rv = rv[tuple(idx)]
                lv = (ltok[:].rearrange("p (ck k) -> p ck k", ck=NCH, k=16)
                      [:, :, i:i + 1])
                for _ in range(len(runs) - 1):
                    lv = lv.unsqueeze(3)
                lv = lv.broadcast_to([128, NCH] + sizes)
                if i == 0:
                    nc.vector.tensor_tensor(gv, lv, rv, ALU.mult)
                else:
                    lsv = natview(lsb_t[:], koff)
                    trv = natview(trm[:], koff)
                    nc.vector.tensor_tensor(lsv, lv, sv, ALU.mult)
                    nc.vector.tensor_tensor(trv, lsv, rv, ALU.mult)
                    nc.vector.tensor_tensor(gv, gv, trv, ALU.add)

            for i in range(16):
                gp_emit(i, 0, 16)

            # gated gelu + W2 + residual -> h_new (own half)
            gate = sb.tile([128, NF], f32, name="gate", bufs=1)
            g4 = gp[:].rearrange("p (c h k) -> p c h k", c=16, h=H, k=16)
            nc.scalar.activation(
                gate[:].rearrange("p (c h) -> p c h", c=16, h=H).unsqueeze(3),
                g4[:, :, :, 0:1], mybir.ActivationFunctionType.Gelu_apprx_tanh)
            z = sb.tile([128, 16 * NF], f32, name="z", bufs=1)
            nc.vector.tensor_tensor(
                z[:].rearrange("p (c h k) -> p c h k", c=16, h=H, k=16), g4,
                gate[:].rearrange("p (c h) -> p c h", c=16, h=H)
                .unsqueeze(3).broadcast_to([128, 16, H, 16]), ALU.mult)
            zT = sb.tile([NF, NOWN], f32, name="zT", bufs=1)
            with tc.tile_pool(name="pzt", bufs=2, space="PSUM") as pzt:
                for c in range(16):
                    pz = pzt.tile([NF, 128], f32, name="pz", bufs=2)
                    nc.tensor.transpose(pz[:], z[:, c * NF:(c + 1) * NF],
                                        idn[:, :])
                    nc.vector.tensor_copy(zT[:, c * 128:(c + 1) * 128], pz[:])
            h_new = sb.tile([NF, NOWN], f32, name="h_new", bufs=1)
            with tc.tile_pool(name="pw2", bufs=2, space="PSUM") as pw2:
                for s in range(NOWN // 512):
                    sl = slice(s * 512, (s + 1) * 512)
                    pm = pw2.tile([NF, 512], f32, name="pm", bufs=2)
                    nc.tensor.matmul(pm[:], R(wsb[f"w2{l}"][:]), R(zT[:, sl]),
                                     start=True, stop=True)
                    nc.vector.tensor_tensor(h_new[:, sl], pm[:], h_att[:, sl],
                                            ALU.add)

            if l < NBLK - 1:
                # AllGather own half -> full h for next layer
                with tc.tile_pool(name="dcc", bufs=1, space="DRAM") as dcc:
                    ccin = dcc.tile([NF, NOWN], f32, name="ccin")
                    ccout = dcc.tile([2 * NF, NOWN], f32, name="ccout")
                    nc.sync.dma_start(ccin[:], h_new[:])
                    nc.gpsimd.collective_compute(
                        "AllGather", mybir.AluOpType.bypass,
                        replica_groups=[[0, 1], [2, 3], [4, 5], [6, 7]],
                        ins=[ccin[:]], outs=[ccout[:]])
                    h_next = sb.tile([NF, N], f32, name="hT", tag="hT", bufs=1)
                    nc.sync.dma_start(h_next[:, 0:NOWN], ccout[0:NF, :])
                    nc.sync.dma_start(h_next[:, NOWN:N], ccout[NF:2 * NF, :])
                h_cur = h_next
            else:
                h_cur = h_new  # [80, 2048] own half only

        # ---------------- output projection (own half, token-major out)
        outT = sb.tile([35, NOWN], f32, name="outT", bufs=1)
        with tc.tile_pool(name="pout", bufs=2, space="PSUM") as pout:
            for s in range(NOWN // 512):
                sl = slice(s * 512, (s + 1) * 512)
                po = pout.tile([35, 512], f32, name="po", bufs=2)
                nc.tensor.matmul(po[:], R(wsb["wsel"][:]), R(h_cur[:, sl]),
                                 start=True, stop=True)
                nc.vector.tensor_copy(outT[:, sl], po[:])
            for c in range(NOWN // 128):
                pot = pout.tile([128, 35], f32, name="pot", bufs=2)
                nc.tensor.transpose(pot[:], outT[:, c * 128:(c + 1) * 128],
                                    idn[:35, :35])
                osb = sb.tile([128, 35], f32, name="osb", bufs=3)
                nc.vector.tensor_copy(osb[:], pot[:])
                nc.sync.dma_start(out_p[c * 128:(c + 1) * 128, :], osb[:])

        sb_cm.__exit__(None, None, None)
        cst_cm.__exit__(None, None, None)

    nc.compile()
    _split_excess_waits(nc)
    return nc


def _get_built():
    global _BUILT
    if _BUILT is None:
        _BUILT = _build_nc()
    return _BUILT


# ---------------------------------------------------------------- entry point
def kernel(x, w_in, w_out, wq, wk, wv, wo, w_mlp1, w_mlp2):
    from concourse.bass_utils import run_bass_kernel_spmd

    x = np.asarray(x, np.float32)
    consts = _build_consts(
        np.asarray(w_in, np.float32), np.asarray(w_out, np.float32),
        np.asarray(wq, np.float32), np.asarray(wk, np.float32),
        np.asarray(wv, np.float32), np.asarray(wo, np.float32),
        np.asarray(w_mlp1, np.float32), np.asarray(w_mlp2, np.float32))
    in_maps = []
    for i in range(8):
        m = dict(consts)
        m["x"] = np.ascontiguousarray(x[i // 2])
        in_maps.append(m)
    nc = _get_built()
    res = run_bass_kernel_spmd(nc, in_maps, core_ids=list(range(8)))
    out = np.zeros((B, N, 35), np.float32)
    for i in range(8):
        half = i % 2
        out[i // 2, half * NOWN:(half + 1) * NOWN, :] = res.results[i]["out"]
    return out

